# revision 19
# baseline (speedup 1.0000x reference)
"""Trainium2 Bass kernel for channel-attention (XCA-style) nn.Module.

Per batch (8 batches -> 8 NeuronCores, pure data parallel):
  qkv = w_qkv @ x            (1x1 conv, 192 -> 576 channels)
  qkv = dwconv3x3(qkv)       (depthwise, per-channel 3x3, zero pad)
  q,k,v = split(qkv); per head (4 heads, 48 ch):
  score = softmax((q/||q||) @ (k/||k||)^T * temp)   contracting hw=16384
  out   = w_proj @ (score @ v)

Design notes (per core):
 - qkv matmul on PE in float32r (1 cyc/row at N>=256), streamed in
   chunks of 16 image rows (+1 halo row each side, recomputed).
 - qkv psum evicted as bf16 into a ring laid out with row stride 132
   (128 cols + 4 zero gap cols): depthwise taps become shifted APs with
   correct zero padding at image edges.  ring B = ring A shifted +1
   element (SBUF->SBUF DMA) keeps dx=+-1 taps 4-byte aligned for DVE
   perf modes.
 - depthwise 3x3: per 128-channel block, fused MAC chain on DVE
   (scalar_tensor_tensor) or mul/add split (configurable).
 - q,k chunks DMA-transposed (xbar) to [hw, ch]; score^T accumulated
   across all chunks in 2 persistent PSUM banks (2 head pairs).
 - L2 norms via ACT Square+accum_out; temperature and 1/||.|| folded
   into the [96,96] score evictions; per-head softmax; probs
   transposed on PE; AV + proj matmuls stream over hw tiles; v spilled
   to DRAM bf16 between stages to fit SBUF.
"""

import sys

sys.path.insert(0, "/opt/trn_rl_repo")

import numpy as np
import ml_dtypes

import concourse.bass as bass
import concourse.mybir as mybir
import concourse.tile as tile
from concourse import bacc
from concourse.bass import ts, ds
from concourse.bass_utils import run_bass_kernel_spmd
from concourse.masks import make_identity

F32 = mybir.dt.float32
F32R = mybir.dt.float32r
BF16 = mybir.dt.bfloat16

DIM = 192
NH = 4
CH = DIM // NH  # 48
C3 = 3 * DIM  # 576
H = 128
W = 128
HW = H * W
B = 8

NPB = 5  # qkv channel partition blocks: 4x128 + 64
PB_SZ = [128, 128, 128, 128, 64]
CHUNK = 16  # image rows per chunk
NCHUNK = H // CHUNK
RROWS = CHUNK + 2  # ring rows = chunk + halo
RSTR = 132  # ring row stride in elements
NTPC = (W * CHUNK) // 128  # 128-px chunklets per chunk

MUL = mybir.AluOpType.mult
ADD = mybir.AluOpType.add
AF = mybir.ActivationFunctionType
AX = mybir.AxisListType

DW_MODE = "split"  # "stt" = fused 9-pass MAC chain; "split" = 17-pass mul/add


def build(dw_mode=None):
    dw_mode = dw_mode or DW_MODE
    nc = bacc.Bacc(None, target_bir_lowering=False)

    xd = nc.dram_tensor("x", [DIM, HW], BF16, kind="ExternalInput")
    wqd = nc.dram_tensor("wq", [128, 2, C3], BF16, kind="ExternalInput")
    wpd = nc.dram_tensor("wp", [96, 2, DIM], BF16, kind="ExternalInput")
    dwd = nc.dram_tensor("dww", [128, NPB, 9], F32, kind="ExternalInput")
    tvd = nc.dram_tensor("tmpv", [96, 2], F32, kind="ExternalInput")
    mkd = nc.dram_tensor("mask", [96, 96], F32, kind="ExternalInput")
    isd = nc.dram_tensor("idshift", [128, 64], BF16, kind="ExternalInput")
    outd = nc.dram_tensor("out", [DIM, HW], F32, kind="ExternalOutput")
    # v spill scratch (bf16), head-pair split: ch 0..95 and ch 96..191
    vda = nc.dram_tensor("vsa", [96, HW], BF16, kind="Internal")
    vdb = nc.dram_tensor("vsb", [96, HW], BF16, kind="Internal")

    with tile.TileContext(nc) as tc:
        _body(nc, tc, xd, wqd, wpd, dwd, tvd, mkd, isd, outd, vda, vdb, dw_mode)
    nc.compile()
    return nc


def _body(nc, tc, xd, wqd, wpd, dwd, tvd, mkd, isd, outd, vda, vdb, dw_mode):
    import contextlib

    xr = xd

    with contextlib.ExitStack() as ctx:
        consts = ctx.enter_context(tc.tile_pool(name="consts", bufs=1))
        smx = ctx.enter_context(tc.tile_pool(name="smx", bufs=1))

        # ---------------- constants ----------------
        wq = consts.tile([128, 2, C3], BF16, tag="wq")
        nc.sync.dma_start(wq[:], wqd[:, :, :])
        wp = consts.tile([96, 2, DIM], BF16, tag="wp")
        nc.sync.dma_start(wp[:], wpd[:, :, :])
        dww = consts.tile([128, NPB, 9], F32, tag="dww")
        nc.sync.dma_start(dww[:], dwd[:, :, :])
        tmpv = consts.tile([96, 2], F32, tag="tmpv")
        nc.sync.dma_start(tmpv[:], tvd[:, :])
        mask = consts.tile([96, 96], F32, tag="mask")
        nc.sync.dma_start(mask[:], mkd[:, :])
        ident = consts.tile([128, 128], F32, tag="ident")
        make_identity(nc, ident[:])
        identb = consts.tile([128, 128], BF16, tag="identb")
        make_identity(nc, identb[:])
        idsh = consts.tile([128, 64], BF16, tag="idsh")
        nc.sync.dma_start(idsh[:], isd[:, :])


        # ============ stage A: qkv + dw + norms + score^T ============
        with contextlib.ExitStack() as sa:
            ringp = sa.enter_context(tc.tile_pool(name="ring", bufs=2))
            xp = sa.enter_context(tc.tile_pool(name="xp", bufs=2))
            pssc = sa.enter_context(
                tc.tile_pool(name="pssc", bufs=1, space=bass.MemorySpace.PSUM)
            )
            psnrm = sa.enter_context(
                tc.tile_pool(name="psnrm", bufs=1, space=bass.MemorySpace.PSUM)
            )
            sb = contextlib.ExitStack()
            psqkv = sb.enter_context(
                tc.tile_pool(name="psqkv", bufs=2, space=bass.MemorySpace.PSUM)
            )
            tpsp = sb.enter_context(
                tc.tile_pool(name="tps", bufs=1, space=bass.MemorySpace.PSUM)
            )
            dwt = sa.enter_context(tc.tile_pool(name="dwt", bufs=2))
            qkp = sa.enter_context(tc.tile_pool(name="qkp", bufs=2))
            qtp = sa.enter_context(tc.tile_pool(name="qtp", bufs=2))
            vst = sa.enter_context(tc.tile_pool(name="vst", bufs=2))
            nrm = sa.enter_context(tc.tile_pool(name="nrm", bufs=2))

            scps = [pssc.tile([96, 96], F32, tag=f"sc{i}", name=f"scps{i}") for i in range(2)]
            nqps = psnrm.tile([96, 192], F32, tag="nq")
            nkps = psnrm.tile([96, 192], F32, tag="nk")

            for c in range(NCHUNK):
                r0 = c * CHUNK - 1  # raw image row held by ring row 0
                row_lo = 1 if c == 0 else 0
                row_hi = RROWS - 1 if c == NCHUNK - 1 else RROWS
                nrows = row_hi - row_lo
                npix = nrows * W
                base_px = (r0 + row_lo) * W

                # ring tiles for this chunk (per pblock)
                rA = [
                    ringp.tile([128, RROWS, RSTR], BF16, tag=f"rA{pb}", name=f"rA{pb}_{c}")
                    for pb in range(NPB)
                ]
                rB = [
                    ringp.tile([128, RROWS, RSTR], BF16, tag=f"rB{pb}", name=f"rB{pb}_{c}")
                    for pb in range(NPB)
                ]
                for pb in range(NPB):
                    # zero the gap columns (stale from slot reuse)
                    nc.vector.memset(rA[pb][:, :, 128:132], 0.0)
                    if c == 0:
                        nc.vector.memset(rA[pb][:, 0, :], 0.0)
                    if c == NCHUNK - 1:
                        nc.vector.memset(rA[pb][:, RROWS - 1, :], 0.0)

                # --- x in + qkv matmul + evict to ring A ---
                nt = (npix + 511) // 512
                for j in range(nt):
                    w0 = j * 512
                    wn = min(512, npix - w0)
                    xt = xp.tile([128, 2, 512], BF16, tag="xt")
                    nc.sync.dma_start(
                        xt[:, 0, :wn], xr[0:128, ds(base_px + w0, wn)]
                    )
                    nc.sync.dma_start(
                        xt[0:64, 1, :wn], xr[128:192, ds(base_px + w0, wn)]
                    )
                    for mb in range(NPB):
                        msz = PB_SZ[mb]
                        ps = psqkv.tile([128, 512], F32, tag="qkvps")
                        nc.tensor.matmul(
                            ps[:msz, :wn],
                            wq[:, 0, ds(mb * 128, msz)],
                            xt[:, 0, :wn],
                            start=True,
                            stop=False,
                        )
                        nc.tensor.matmul(
                            ps[:msz, :wn],
                            wq[0:64, 1, ds(mb * 128, msz)],
                            xt[0:64, 1, :wn],
                            start=False,
                            stop=True,
                        )
                        rr = row_lo + (w0 // 128)
                        nr = wn // 128
                        dst = rA[mb][:msz, rr : rr + nr, 0:128]
                        src = ps[:msz, :wn].rearrange("p (r w) -> p r w", w=128)
                        nc.scalar.copy(dst, src)

                # --- ring B = ring A shifted one element; fix first elem ---
                nel = RROWS * RSTR
                for pb in range(NPB):
                    av = rA[pb][:].rearrange("p r s -> p (r s)")
                    bv = rB[pb][:].rearrange("p r s -> p (r s)")
                    nc.sync.dma_start(bv[:, 1:nel], av[:, 0 : nel - 1])
                    nc.vector.memset(rB[pb][:, 0, 0:1], 0.0)

                # --- depthwise 3x3 ---
                qk = qkp.tile([128, 3, CHUNK * W], BF16, tag="qk")
                va = vst.tile([128, CHUNK * W], BF16, tag="va")
                vb = vst.tile([64, CHUNK * W], BF16, tag="vb")
                for pb in range(NPB):
                    psz = PB_SZ[pb]
                    if pb < 3:
                        dest = qk[:psz, pb, :]
                    elif pb == 3:
                        dest = va[:, :]
                    else:
                        dest = vb[:, :]
                    dest3 = dest.rearrange("p (r w) -> p r w", w=128)

                    def tap(dy, dx):
                        if dx == 0:
                            return rA[pb][:psz, 1 + dy : 1 + dy + CHUNK, 0:128]
                        return rB[pb][
                            :psz, 1 + dy : 1 + dy + CHUNK, 1 + dx : 129 + dx
                        ]

                    taps = [(dy, dx) for dy in (-1, 0, 1) for dx in (-1, 0, 1)]

                    if dw_mode == "stt":
                        # m = t0*w0; then acc = t_i*w_i + acc (fused)
                        prev = None
                        for i, (dy, dx) in enumerate(taps):
                            wsc = dww[:psz, pb, i : i + 1]
                            last = i == 8
                            if i == 0:
                                a = dwt.tile([128, CHUNK, W], BF16, tag="a0")
                                nc.vector.tensor_scalar(
                                    a[:psz], tap(dy, dx), wsc, None, op0=MUL
                                )
                                prev = a
                            else:
                                o3 = (
                                    dest3
                                    if last
                                    else dwt.tile(
                                        [128, CHUNK, W], BF16, tag=f"a{i % 2}"
                                    )
                                )
                                oap = o3 if last else o3[:psz]
                                nc.vector.scalar_tensor_tensor(
                                    oap,
                                    tap(dy, dx),
                                    wsc,
                                    prev[:psz],
                                    op0=MUL,
                                    op1=ADD,
                                )
                                if not last:
                                    prev = o3
                    else:
                        prev = None
                        for i, (dy, dx) in enumerate(taps):
                            wsc = dww[:psz, pb, i : i + 1]
                            m = dwt.tile([128, CHUNK, W], BF16, tag=f"m{i % 2}")
                            if i <= 1:
                                nc.scalar.activation(
                                    m[:psz], tap(dy, dx), AF.Copy, scale=wsc
                                )
                            else:
                                nc.vector.tensor_scalar(
                                    m[:psz], tap(dy, dx), wsc, None, op0=MUL
                                )
                            if i == 0:
                                prev = m
                                continue
                            last = i == 8
                            o3 = (
                                dest3
                                if last
                                else dwt.tile(
                                    [128, CHUNK, W], BF16, tag=f"a{i % 2}"
                                )
                            )
                            oap = o3 if last else o3[:psz]
                            nc.vector.tensor_tensor(
                                oap, prev[:psz], m[:psz], op=ADD
                            )
                            if not last:
                                prev = o3

                # --- spill v chunk to DRAM (pair-split) ---
                csl = ds(c * CHUNK * W, CHUNK * W)
                nc.sync.dma_start(vda[:, csl], va[0:96, :])
                nc.sync.dma_start(vdb[0:32, csl], va[96:128, :])
                nc.sync.dma_start(vdb[32:96, csl], vb[:])

                # --- transpose q,k + score matmuls ---
                qt = qtp.tile([128, NTPC, DIM], BF16, tag="qt")
                kt = qtp.tile([128, NTPC, DIM], BF16, tag="kt")
                for ii in range(0, NTPC, 2):
                    tq = tpsp.tile([128, 2, 192], BF16, tag="tq")
                    tk = tpsp.tile([128, 2, 192], BF16, tag="tk")
                    for u in range(2):
                        i = ii + u
                        nc.tensor.transpose(
                            tq[:, u, 0:128], qk[:, 0, ts(i, 128)], identb[:]
                        )
                        nc.tensor.transpose(
                            tq[:, u, 128:192],
                            qk[0:64, 1, ts(i, 128)],
                            identb[0:64, 0:64],
                        )
                        nc.tensor.transpose(
                            tk[:, u, 0:64],
                            qk[64:128, 1, ts(i, 128)],
                            idsh[64:128, :],
                        )
                        nc.tensor.transpose(
                            tk[:, u, 64:192], qk[:, 2, ts(i, 128)], identb[:]
                        )
                    nc.scalar.copy(qt[:, ii : ii + 2, :], tq[:])
                    nc.scalar.copy(kt[:, ii : ii + 2, :], tk[:])
                for i in range(NTPC):
                    first = c == 0 and i == 0
                    last = c == NCHUNK - 1 and i == NTPC - 1
                    nc.tensor.matmul(
                        scps[0][:],
                        kt[:, i, 0:96],
                        qt[:, i, 0:96],
                        start=first,
                        stop=last,
                    )
                    nc.tensor.matmul(
                        scps[1][:],
                        kt[:, i, 96:192],
                        qt[:, i, 96:192],
                        start=first,
                        stop=last,
                    )
                    for g in range(2):
                        nc.tensor.matmul(
                            nqps[:, ts(g, 96)],
                            qt[:, i, ts(g, 96)],
                            qt[:, i, ts(g, 96)],
                            start=first,
                            stop=last,
                        )
                        nc.tensor.matmul(
                            nkps[:, ts(g, 96)],
                            kt[:, i, ts(g, 96)],
                            kt[:, i, ts(g, 96)],
                            start=first,
                            stop=last,
                        )

            sb.close()
            psB = sa.enter_context(
                tc.tile_pool(name="psB", bufs=1, space=bass.MemorySpace.PSUM)
            )
            # ---------- score finalize + softmax ----------
            # extract Gram diagonals -> per-pair scale vectors
            rsq = smx.tile([96, 2], F32, tag="rsq")
            rsk = smx.tile([96, 2], F32, tag="rsk")
            for g in range(2):
                for ps_, dst in ((nqps, rsq), (nkps, rsk)):
                    dtmp = smx.tile([96, 96], F32, tag="dtmp", name=f"dt{g}")
                    nc.vector.tensor_tensor(
                        dtmp[:], ps_[:, ts(g, 96)], ident[0:96, 0:96], op=MUL
                    )
                    nc.vector.reduce_sum(
                        dst[:, g : g + 1], dtmp[:], axis=AX.X
                    )
            for t_ in (rsq, rsk):
                nc.scalar.activation(t_[:], t_[:], AF.Sqrt)
                nc.vector.tensor_scalar(
                    t_[:], t_[:], 1e-12, None, op0=mybir.AluOpType.max
                )
                nc.vector.reciprocal(t_[:], t_[:])
            nc.vector.tensor_tensor(rsq[:], rsq[:], tmpv[:], op=MUL)
            rsq_a, rsq_b = rsq[:, 0:1], rsq[:, 1:2]
            rsk_a, rsk_b = rsk[:, 0:1], rsk[:, 1:2]

            sc_t = smx.tile([96, 2, 96], F32, tag="sct")
            nc.scalar.activation(
                sc_t[:, 0, :], scps[0][:], AF.Copy, scale=rsk_a
            )
            nc.scalar.activation(
                sc_t[:, 1, :], scps[1][:], AF.Copy, scale=rsk_b
            )
            scp2 = [psB.tile([96, 96], F32, tag=f"sc2_{i}", name=f"scp2_{i}") for i in range(2)]
            nc.tensor.transpose(scp2[0][:], sc_t[:, 0, :], ident[0:96, 0:96])
            nc.tensor.transpose(scp2[1][:], sc_t[:, 1, :], ident[0:96, 0:96])

            # evict full rows with q-scale, then add -1e30 off-diag mask so
            # the full-row softmax ignores cross-head blocks
            sc = smx.tile([96, 2, 96], F32, tag="sc")
            for g in range(2):
                qsc = rsq_a if g == 0 else rsq_b
                nc.scalar.activation(
                    sc[:, g, :], scp2[g][:], AF.Copy, scale=qsc
                )
                nc.vector.tensor_tensor(
                    sc[:, g, :], sc[:, g, :], mask[:], op=ADD
                )

            probs = smx.tile([96, 2, 96], F32, tag="probs")
            for g in range(2):
                mx = smx.tile([96, 1], F32, tag=f"mx{g}", name=f"mx{g}")
                nc.vector.reduce_max(mx[:], sc[:, g, :], axis=AX.X)
                nmx = smx.tile([96, 1], F32, tag=f"nmx{g}", name=f"nmx{g}")
                nc.vector.tensor_scalar(nmx[:], mx[:], -1.0, None, op0=MUL)
                e = smx.tile([96, 96], F32, tag=f"e{g}", name=f"e{g}")
                nc.scalar.activation(e[:], sc[:, g, :], AF.Exp, bias=nmx[:])
                sm = smx.tile([96, 1], F32, tag=f"sm{g}", name=f"sm{g}")
                nc.vector.reduce_sum(sm[:], e[:], axis=AX.X)
                nc.vector.reciprocal(sm[:], sm[:])
                nc.vector.tensor_scalar(
                    probs[:, g, :], e[:], sm[:], None, op0=MUL
                )

            pT = smx.tile([96, 2, 96], BF16, tag="pT")
            for g in range(2):
                pp = psB.tile([96, 96], F32, tag=f"pT{g}")
                nc.tensor.transpose(pp[:], probs[:, g, :], ident[0:96, 0:96])
                nc.vector.tensor_copy(pT[:, g, :], pp[:])

        # ============ stage C: AV + proj, streaming over hw ============
        with (
            tc.tile_pool(name="psav", bufs=2, space=bass.MemorySpace.PSUM) as psav,
            tc.tile_pool(name="pspj", bufs=2, space=bass.MemorySpace.PSUM) as pspj,
            tc.tile_pool(name="vload", bufs=3) as vload,
            tc.tile_pool(name="attn", bufs=2) as attnp,
            tc.tile_pool(name="outp", bufs=2) as outp,
        ):
            for j in range(HW // 512):
                sl = ts(j, 512)
                vla = vload.tile([96, 512], BF16, tag="vla")
                vlb = vload.tile([96, 512], BF16, tag="vlb")
                nc.sync.dma_start(vla[:], vda[:, sl])
                nc.sync.dma_start(vlb[:], vdb[:, sl])

                pa = psav.tile([96, 512], F32, tag="pa")
                pb_ = psav.tile([96, 512], F32, tag="pb")
                # probs off-diagonal blocks are zero -> block-diag K=96 matmul
                nc.tensor.matmul(
                    pa[:], pT[:, 0, :], vla[:], start=True, stop=True
                )
                nc.tensor.matmul(
                    pb_[:], pT[:, 1, :], vlb[:], start=True, stop=True
                )
                at = attnp.tile([96, 2, 512], BF16, tag="at")
                nc.vector.tensor_copy(at[:, 0, :], pa[:])
                nc.vector.tensor_copy(at[:, 1, :], pb_[:])

                p0 = pspj.tile([128, 512], F32, tag="p0")
                p1 = pspj.tile([64, 512], F32, tag="p1")
                nc.tensor.matmul(
                    p0[:], wp[:, 0, 0:128], at[:, 0, :], start=True, stop=False
                )
                nc.tensor.matmul(
                    p0[:], wp[:, 1, 0:128], at[:, 1, :], start=False, stop=True
                )
                nc.tensor.matmul(
                    p1[:], wp[:, 0, 128:192], at[:, 0, :], start=True, stop=False
                )
                nc.tensor.matmul(
                    p1[:], wp[:, 1, 128:192], at[:, 1, :], start=False, stop=True
                )
                ot = outp.tile([128, 512], F32, tag="ot")
                ot2 = outp.tile([64, 512], F32, tag="ot2")
                nc.vector.tensor_copy(ot[:], p0[:])
                nc.scalar.copy(ot2[:], p1[:])
                nc.sync.dma_start(outd[0:128, sl], ot[:])
                nc.sync.dma_start(outd[128:192, sl], ot2[:])


_NC_CACHE = {}


def _get_nc(dw_mode=None):
    key = dw_mode or DW_MODE
    if key not in _NC_CACHE:
        _NC_CACHE[key] = build(key)
    return _NC_CACHE[key]


def prep_inputs(x, w_qkv, w_dw, w_proj, temperature):
    x = np.asarray(x, np.float32)
    w_qkv = np.asarray(w_qkv, np.float32)
    w_dw = np.asarray(w_dw, np.float32).reshape(C3, 9)
    w_proj = np.asarray(w_proj, np.float32)
    temperature = np.asarray(temperature, np.float32).reshape(NH)

    wqT = np.ascontiguousarray(w_qkv.T)  # [192, 576]
    wq = np.zeros((128, 2, C3), np.float32)
    wq[:, 0, :] = wqT[0:128]
    wq[0:64, 1, :] = wqT[128:192]
    wq = wq.astype(ml_dtypes.bfloat16)

    wpT = np.ascontiguousarray(w_proj.T)  # [c, o]
    wp = np.zeros((96, 2, DIM), np.float32)
    wp[:, 0, :] = wpT[0:96]
    wp[:, 1, :] = wpT[96:192]
    wp = wp.astype(ml_dtypes.bfloat16)

    dww = np.zeros((128, NPB, 9), np.float32)
    for pb in range(NPB):
        sz = PB_SZ[pb]
        dww[:sz, pb, :] = w_dw[pb * 128 : pb * 128 + sz]

    idshift = np.zeros((128, 64), np.float32)
    idshift[64:128, :] = np.eye(64)
    idshift = idshift.astype(ml_dtypes.bfloat16)

    mask = np.full((96, 96), -1e30, np.float32)
    mask[0:48, 0:48] = 0.0
    mask[48:96, 48:96] = 0.0

    tmpv = np.ones((96, 2), np.float32)
    tmpv[:, 0] = temperature[np.arange(96) // CH]
    tmpv[:, 1] = temperature[(96 + np.arange(96)) // CH]

    maps = []
    for b in range(B):
        maps.append(
            {
                "x": np.ascontiguousarray(x[b].reshape(DIM, HW)).astype(ml_dtypes.bfloat16),
                "wq": wq,
                "wp": wp,
                "dww": dww,
                "tmpv": tmpv,
                "mask": mask,
                "idshift": idshift,
            }
        )
    return maps


def kernel(x, w_qkv, w_dw, w_proj, temperature, trace=False, tmpdir=None,
           dw_mode=None):
    nc = _get_nc(dw_mode)
    maps = prep_inputs(x, w_qkv, w_dw, w_proj, temperature)
    res = run_bass_kernel_spmd(
        nc, maps, core_ids=list(range(B)), trace=trace, tmpdir=tmpdir
    )
    out = np.stack(
        [np.asarray(r["out"]).reshape(DIM, H, W) for r in res.results]
    ).astype(np.float32)
    kernel.last_exec_time_ns = res.exec_time_ns
    return out


if __name__ == "__main__":
    nc = build()
    print("build ok")


# revision 21
# speedup vs baseline: 1.0374x; 1.0374x over previous
"""Trainium2 Bass kernel for channel-attention (XCA-style) nn.Module.

Per batch (8 batches -> 8 NeuronCores, pure data parallel):
  qkv = w_qkv @ x            (1x1 conv, 192 -> 576 channels)
  qkv = dwconv3x3(qkv)       (depthwise, per-channel 3x3, zero pad)
  q,k,v = split(qkv); per head (4 heads, 48 ch):
  score = softmax((q/||q||) @ (k/||k||)^T * temp)   contracting hw=16384
  out   = w_proj @ (score @ v)

Design notes (per core):
 - qkv matmul on PE in float32r (1 cyc/row at N>=256), streamed in
   chunks of 16 image rows (+1 halo row each side, recomputed).
 - qkv psum evicted as bf16 into a ring laid out with row stride 132
   (128 cols + 4 zero gap cols): depthwise taps become shifted APs with
   correct zero padding at image edges.  ring B = ring A shifted +1
   element (SBUF->SBUF DMA) keeps dx=+-1 taps 4-byte aligned for DVE
   perf modes.
 - depthwise 3x3: per 128-channel block, fused MAC chain on DVE
   (scalar_tensor_tensor) or mul/add split (configurable).
 - q,k chunks DMA-transposed (xbar) to [hw, ch]; score^T accumulated
   across all chunks in 2 persistent PSUM banks (2 head pairs).
 - L2 norms via ACT Square+accum_out; temperature and 1/||.|| folded
   into the [96,96] score evictions; per-head softmax; probs
   transposed on PE; AV + proj matmuls stream over hw tiles; v spilled
   to DRAM bf16 between stages to fit SBUF.
"""

import sys

sys.path.insert(0, "/opt/trn_rl_repo")

import numpy as np
import ml_dtypes

import concourse.bass as bass
import concourse.mybir as mybir
import concourse.tile as tile
from concourse import bacc
from concourse.bass import ts, ds
from concourse.bass_utils import run_bass_kernel_spmd
from concourse.masks import make_identity

F32 = mybir.dt.float32
F32R = mybir.dt.float32r
BF16 = mybir.dt.bfloat16

DIM = 192
NH = 4
CH = DIM // NH  # 48
C3 = 3 * DIM  # 576
H = 128
W = 128
HW = H * W
B = 8

NPB = 5  # qkv channel partition blocks: 4x128 + 64
PB_SZ = [128, 128, 128, 128, 64]
CHUNK = 16  # image rows per chunk
NCHUNK = H // CHUNK
RROWS = CHUNK + 2  # ring rows = chunk + halo
RSTR = 132  # ring row stride in elements
NTPC = (W * CHUNK) // 128  # 128-px chunklets per chunk

MUL = mybir.AluOpType.mult
ADD = mybir.AluOpType.add
AF = mybir.ActivationFunctionType
AX = mybir.AxisListType

DW_MODE = "split"  # "stt" = fused 9-pass MAC chain; "split" = 17-pass mul/add


def build(dw_mode=None):
    dw_mode = dw_mode or DW_MODE
    nc = bacc.Bacc(None, target_bir_lowering=False)

    xd = nc.dram_tensor("x", [DIM, HW], BF16, kind="ExternalInput")
    wqd = nc.dram_tensor("wq", [128, 2, C3], BF16, kind="ExternalInput")
    wpd = nc.dram_tensor("wp", [96, 2, DIM], BF16, kind="ExternalInput")
    dwd = nc.dram_tensor("dww", [128, NPB, 9], F32, kind="ExternalInput")
    tvd = nc.dram_tensor("tmpv", [128, 3], F32, kind="ExternalInput")
    mkd = nc.dram_tensor("mask", [96, 96], F32, kind="ExternalInput")
    isd = nc.dram_tensor("idshift", [128, 64], BF16, kind="ExternalInput")
    outd = nc.dram_tensor("out", [DIM, HW], F32, kind="ExternalOutput")
    # v spill scratch (bf16), head-pair split: ch 0..95 and ch 96..191
    vda = nc.dram_tensor("vsa", [96, HW], BF16, kind="Internal")
    vdb = nc.dram_tensor("vsb", [96, HW], BF16, kind="Internal")

    with tile.TileContext(nc) as tc:
        _body(nc, tc, xd, wqd, wpd, dwd, tvd, mkd, isd, outd, vda, vdb, dw_mode)
    nc.compile()
    return nc


def _body(nc, tc, xd, wqd, wpd, dwd, tvd, mkd, isd, outd, vda, vdb, dw_mode):
    import contextlib

    xr = xd

    with contextlib.ExitStack() as ctx:
        consts = ctx.enter_context(tc.tile_pool(name="consts", bufs=1))
        smx = ctx.enter_context(tc.tile_pool(name="smx", bufs=1))

        # ---------------- constants ----------------
        wq = consts.tile([128, 2, C3], BF16, tag="wq")
        nc.sync.dma_start(wq[:], wqd[:, :, :])
        wp = consts.tile([96, 2, DIM], BF16, tag="wp")
        nc.sync.dma_start(wp[:], wpd[:, :, :])
        dww = consts.tile([128, NPB, 9], F32, tag="dww")
        nc.sync.dma_start(dww[:], dwd[:, :, :])
        tmpv = consts.tile([128, 3], F32, tag="tmpv")
        nc.sync.dma_start(tmpv[:], tvd[:, :])
        mask = consts.tile([96, 96], F32, tag="mask")
        nc.sync.dma_start(mask[:], mkd[:, :])
        ident = consts.tile([128, 128], F32, tag="ident")
        make_identity(nc, ident[:])
        identb = consts.tile([128, 128], BF16, tag="identb")
        make_identity(nc, identb[:])
        idsh = consts.tile([128, 64], BF16, tag="idsh")
        nc.sync.dma_start(idsh[:], isd[:, :])
        n2 = consts.tile([128, 3], F32, tag="n2")
        nc.vector.memset(n2[:], 0.0)

        # ============ stage A: qkv + dw + norms + score^T ============
        with contextlib.ExitStack() as sa:
            ringp = sa.enter_context(tc.tile_pool(name="ring", bufs=2))
            xp = sa.enter_context(tc.tile_pool(name="xp", bufs=2))
            pssc = sa.enter_context(
                tc.tile_pool(name="pssc", bufs=1, space=bass.MemorySpace.PSUM)
            )
            sb = contextlib.ExitStack()
            psqkv = sb.enter_context(
                tc.tile_pool(name="psqkv", bufs=2, space=bass.MemorySpace.PSUM)
            )
            tpsp = sb.enter_context(
                tc.tile_pool(name="tps", bufs=2, space=bass.MemorySpace.PSUM)
            )
            dwt = sa.enter_context(tc.tile_pool(name="dwt", bufs=2))
            qkp = sa.enter_context(tc.tile_pool(name="qkp", bufs=2))
            qtp = sa.enter_context(tc.tile_pool(name="qtp", bufs=2))
            vst = sa.enter_context(tc.tile_pool(name="vst", bufs=2))
            nrm = sa.enter_context(tc.tile_pool(name="nrm", bufs=2))

            scps = [pssc.tile([96, 96], F32, tag=f"sc{i}", name=f"scps{i}") for i in range(2)]

            for c in range(NCHUNK):
                r0 = c * CHUNK - 1  # raw image row held by ring row 0
                row_lo = 1 if c == 0 else 0
                row_hi = RROWS - 1 if c == NCHUNK - 1 else RROWS
                nrows = row_hi - row_lo
                npix = nrows * W
                base_px = (r0 + row_lo) * W

                # ring tiles for this chunk (per pblock)
                rA = [
                    ringp.tile([128, RROWS, RSTR], BF16, tag=f"rA{pb}", name=f"rA{pb}_{c}")
                    for pb in range(NPB)
                ]
                rB = [
                    ringp.tile([128, RROWS, RSTR], BF16, tag=f"rB{pb}", name=f"rB{pb}_{c}")
                    for pb in range(NPB)
                ]
                for pb in range(NPB):
                    # zero gap columns once per physical slot (bufs=2):
                    # evictions never write cols 128..131, so slots stay
                    # zeroed across reuse after chunks 0 and 1
                    if c < 2:
                        nc.vector.memset(rA[pb][:, :, 128:132], 0.0)
                    if c == 0:
                        nc.vector.memset(rA[pb][:, 0, :], 0.0)
                    if c == NCHUNK - 1:
                        nc.vector.memset(rA[pb][:, RROWS - 1, :], 0.0)

                # --- x in + qkv matmul + evict to ring A ---
                nt = (npix + 511) // 512
                for j in range(nt):
                    w0 = j * 512
                    wn = min(512, npix - w0)
                    xt = xp.tile([128, 2, 512], BF16, tag="xt")
                    nc.sync.dma_start(
                        xt[:, 0, :wn], xr[0:128, ds(base_px + w0, wn)]
                    )
                    nc.sync.dma_start(
                        xt[0:64, 1, :wn], xr[128:192, ds(base_px + w0, wn)]
                    )
                    for mb in range(NPB):
                        msz = PB_SZ[mb]
                        ps = psqkv.tile([128, 512], F32, tag="qkvps")
                        nc.tensor.matmul(
                            ps[:msz, :wn],
                            wq[:, 0, ds(mb * 128, msz)],
                            xt[:, 0, :wn],
                            start=True,
                            stop=False,
                        )
                        nc.tensor.matmul(
                            ps[:msz, :wn],
                            wq[0:64, 1, ds(mb * 128, msz)],
                            xt[0:64, 1, :wn],
                            start=False,
                            stop=True,
                        )
                        rr = row_lo + (w0 // 128)
                        nr = wn // 128
                        dst = rA[mb][:msz, rr : rr + nr, 0:128]
                        src = ps[:msz, :wn].rearrange("p (r w) -> p r w", w=128)
                        nc.scalar.copy(dst, src)

                # --- ring B = ring A shifted one element; fix first elem ---
                nel = RROWS * RSTR
                for pb in range(NPB):
                    av = rA[pb][:].rearrange("p r s -> p (r s)")
                    bv = rB[pb][:].rearrange("p r s -> p (r s)")
                    nc.sync.dma_start(bv[:, 1:nel], av[:, 0 : nel - 1])
                    if c < 2:
                        # elem 0 never written by the shift-copy; slot
                        # stays zero across reuse
                        nc.vector.memset(rB[pb][:, 0, 0:1], 0.0)

                # --- depthwise 3x3 ---
                qk = qkp.tile([128, 3, CHUNK * W], BF16, tag="qk")
                va = vst.tile([128, CHUNK * W], BF16, tag="va")
                vb = vst.tile([64, CHUNK * W], BF16, tag="vb")
                for pb in range(NPB):
                    psz = PB_SZ[pb]
                    if pb < 3:
                        dest = qk[:psz, pb, :]
                    elif pb == 3:
                        dest = va[:, :]
                    else:
                        dest = vb[:, :]
                    dest3 = dest.rearrange("p (r w) -> p r w", w=128)

                    def tap(dy, dx):
                        if dx == 0:
                            return rA[pb][:psz, 1 + dy : 1 + dy + CHUNK, 0:128]
                        return rB[pb][
                            :psz, 1 + dy : 1 + dy + CHUNK, 1 + dx : 129 + dx
                        ]

                    taps = [(dy, dx) for dy in (-1, 0, 1) for dx in (-1, 0, 1)]

                    if dw_mode == "stt":
                        # m = t0*w0; then acc = t_i*w_i + acc (fused)
                        prev = None
                        for i, (dy, dx) in enumerate(taps):
                            wsc = dww[:psz, pb, i : i + 1]
                            last = i == 8
                            if i == 0:
                                a = dwt.tile([128, CHUNK, W], BF16, tag="a0")
                                nc.vector.tensor_scalar(
                                    a[:psz], tap(dy, dx), wsc, None, op0=MUL
                                )
                                prev = a
                            else:
                                o3 = (
                                    dest3
                                    if last
                                    else dwt.tile(
                                        [128, CHUNK, W], BF16, tag=f"a{i % 2}"
                                    )
                                )
                                oap = o3 if last else o3[:psz]
                                nc.vector.scalar_tensor_tensor(
                                    oap,
                                    tap(dy, dx),
                                    wsc,
                                    prev[:psz],
                                    op0=MUL,
                                    op1=ADD,
                                )
                                if not last:
                                    prev = o3
                    else:
                        prev = None
                        for i, (dy, dx) in enumerate(taps):
                            wsc = dww[:psz, pb, i : i + 1]
                            m = dwt.tile([128, CHUNK, W], BF16, tag=f"m{i % 2}")
                            if i == 0:
                                nc.scalar.activation(
                                    m[:psz], tap(dy, dx), AF.Copy, scale=wsc
                                )
                            else:
                                nc.vector.tensor_scalar(
                                    m[:psz], tap(dy, dx), wsc, None, op0=MUL
                                )
                            if i == 0:
                                prev = m
                                continue
                            last = i == 8
                            o3 = (
                                dest3
                                if last
                                else dwt.tile(
                                    [128, CHUNK, W], BF16, tag=f"a{i % 2}"
                                )
                            )
                            oap = o3 if last else o3[:psz]
                            nc.vector.tensor_tensor(
                                oap, prev[:psz], m[:psz], op=ADD
                            )
                            if not last:
                                prev = o3

                # --- spill v chunk to DRAM (pair-split) ---
                csl = ds(c * CHUNK * W, CHUNK * W)
                nc.sync.dma_start(vda[:, csl], va[0:96, :])
                nc.sync.dma_start(vdb[0:32, csl], va[96:128, :])
                nc.sync.dma_start(vdb[32:96, csl], vb[:])

                # --- norms (q,k pblocks 0..2) ---
                for pb in range(3):
                    sq = dwt.tile([128, CHUNK, W], BF16, tag="sq", bufs=1)
                    part = nrm.tile([128, 1], F32, tag="part")
                    nc.scalar.activation(
                        sq[:].rearrange("p r w -> p (r w)"),
                        qk[:, pb, :],
                        AF.Square,
                        accum_out=part[:],
                    )
                    nc.vector.tensor_tensor(
                        n2[:, pb : pb + 1], n2[:, pb : pb + 1], part[:], op=ADD
                    )

                # --- transpose q,k + score matmuls ---
                qt = qtp.tile([128, NTPC, DIM], BF16, tag="qt")
                kt = qtp.tile([128, NTPC, DIM], BF16, tag="kt")
                for ii in range(0, NTPC, 2):
                    tq = tpsp.tile([128, 2, 192], BF16, tag="tq")
                    tk = tpsp.tile([128, 2, 192], BF16, tag="tk")
                    for u in range(2):
                        i = ii + u
                        nc.tensor.transpose(
                            tq[:, u, 0:128], qk[:, 0, ts(i, 128)], identb[:]
                        )
                        nc.tensor.transpose(
                            tq[:, u, 128:192],
                            qk[0:64, 1, ts(i, 128)],
                            identb[0:64, 0:64],
                        )
                        nc.tensor.transpose(
                            tk[:, u, 0:64],
                            qk[64:128, 1, ts(i, 128)],
                            idsh[64:128, :],
                        )
                        nc.tensor.transpose(
                            tk[:, u, 64:192], qk[:, 2, ts(i, 128)], identb[:]
                        )
                    nc.scalar.copy(qt[:, ii : ii + 2, :], tq[:])
                    nc.scalar.copy(kt[:, ii : ii + 2, :], tk[:])
                for i in range(NTPC):
                    first = c == 0 and i == 0
                    last = c == NCHUNK - 1 and i == NTPC - 1
                    nc.tensor.matmul(
                        scps[0][:],
                        kt[:, i, 0:96],
                        qt[:, i, 0:96],
                        start=first,
                        stop=last,
                    )
                    nc.tensor.matmul(
                        scps[1][:],
                        kt[:, i, 96:192],
                        qt[:, i, 96:192],
                        start=first,
                        stop=last,
                    )

            sb.close()
            psB = sa.enter_context(
                tc.tile_pool(name="psB", bufs=1, space=bass.MemorySpace.PSUM)
            )
            # ---------- score finalize + softmax ----------
            rs = smx.tile([128, 3], F32, tag="rs")
            nc.scalar.activation(rs[:], n2[:], AF.Sqrt)
            nc.vector.tensor_scalar(
                rs[:], rs[:], 1e-12, None, op0=mybir.AluOpType.max
            )
            nc.vector.reciprocal(rs[:], rs[:])
            nc.vector.tensor_tensor(rs[:], rs[:], tmpv[:], op=MUL)

            # partition-aligned scale vectors for score rows
            # pair a rows: q/k ch 0..95 ; pair b rows: q/k ch 96..191
            rsq_b = smx.tile([96, 1], F32, tag="rsqb")
            rsk_a = smx.tile([96, 1], F32, tag="rska")
            rsk_b = smx.tile([96, 1], F32, tag="rskb")
            nc.sync.dma_start(rsq_b[0:32, :], rs[96:128, 0:1])
            nc.sync.dma_start(rsq_b[32:96, :], rs[0:64, 1:2])
            nc.sync.dma_start(rsk_a[0:64, :], rs[64:128, 1:2])
            nc.sync.dma_start(rsk_a[64:96, :], rs[0:32, 2:3])
            nc.sync.dma_start(rsk_b[:, :], rs[32:128, 2:3])
            rsq_a = rs[:, 0:1]

            sc_t = smx.tile([96, 2, 96], F32, tag="sct")
            nc.scalar.activation(
                sc_t[:, 0, :], scps[0][:], AF.Copy, scale=rsk_a[:]
            )
            nc.scalar.activation(
                sc_t[:, 1, :], scps[1][:], AF.Copy, scale=rsk_b[:]
            )
            scp2 = [psB.tile([96, 96], F32, tag=f"sc2_{i}", name=f"scp2_{i}") for i in range(2)]
            nc.tensor.transpose(scp2[0][:], sc_t[:, 0, :], ident[0:96, 0:96])
            nc.tensor.transpose(scp2[1][:], sc_t[:, 1, :], ident[0:96, 0:96])

            # evict full rows with q-scale, then add -1e30 off-diag mask so
            # the full-row softmax ignores cross-head blocks
            sc = smx.tile([96, 2, 96], F32, tag="sc")
            for g in range(2):
                qsc = rsq_a[0:96] if g == 0 else rsq_b[0:96]
                nc.scalar.activation(
                    sc[:, g, :], scp2[g][:], AF.Copy, scale=qsc
                )
                nc.vector.tensor_tensor(
                    sc[:, g, :], sc[:, g, :], mask[:], op=ADD
                )

            probs = smx.tile([96, 2, 96], F32, tag="probs")
            for g in range(2):
                mx = smx.tile([96, 1], F32, tag=f"mx{g}", name=f"mx{g}")
                nc.vector.reduce_max(mx[:], sc[:, g, :], axis=AX.X)
                nmx = smx.tile([96, 1], F32, tag=f"nmx{g}", name=f"nmx{g}")
                nc.vector.tensor_scalar(nmx[:], mx[:], -1.0, None, op0=MUL)
                e = smx.tile([96, 96], F32, tag=f"e{g}", name=f"e{g}")
                nc.scalar.activation(e[:], sc[:, g, :], AF.Exp, bias=nmx[:])
                sm = smx.tile([96, 1], F32, tag=f"sm{g}", name=f"sm{g}")
                nc.vector.reduce_sum(sm[:], e[:], axis=AX.X)
                nc.vector.reciprocal(sm[:], sm[:])
                nc.vector.tensor_scalar(
                    probs[:, g, :], e[:], sm[:], None, op0=MUL
                )

            pT = smx.tile([96, 2, 96], BF16, tag="pT")
            for g in range(2):
                pp = psB.tile([96, 96], F32, tag=f"pT{g}")
                nc.tensor.transpose(pp[:], probs[:, g, :], ident[0:96, 0:96])
                nc.vector.tensor_copy(pT[:, g, :], pp[:])

        # ============ stage C: AV + proj, streaming over hw ============
        with (
            tc.tile_pool(name="psav", bufs=2, space=bass.MemorySpace.PSUM) as psav,
            tc.tile_pool(name="pspj", bufs=2, space=bass.MemorySpace.PSUM) as pspj,
            tc.tile_pool(name="vload", bufs=3) as vload,
            tc.tile_pool(name="attn", bufs=2) as attnp,
            tc.tile_pool(name="outp", bufs=2) as outp,
        ):
            for j in range(HW // 512):
                sl = ts(j, 512)
                vla = vload.tile([96, 512], BF16, tag="vla")
                vlb = vload.tile([96, 512], BF16, tag="vlb")
                nc.sync.dma_start(vla[:], vda[:, sl])
                nc.sync.dma_start(vlb[:], vdb[:, sl])

                pa = psav.tile([96, 512], F32, tag="pa")
                pb_ = psav.tile([96, 512], F32, tag="pb")
                # probs off-diagonal blocks are zero -> block-diag K=96 matmul
                nc.tensor.matmul(
                    pa[:], pT[:, 0, :], vla[:], start=True, stop=True
                )
                nc.tensor.matmul(
                    pb_[:], pT[:, 1, :], vlb[:], start=True, stop=True
                )
                at = attnp.tile([96, 2, 512], BF16, tag="at")
                nc.scalar.copy(at[:, 0, :], pa[:])
                nc.scalar.copy(at[:, 1, :], pb_[:])

                p0 = pspj.tile([128, 512], F32, tag="p0")
                p1 = pspj.tile([64, 512], F32, tag="p1")
                nc.tensor.matmul(
                    p0[:], wp[:, 0, 0:128], at[:, 0, :], start=True, stop=False
                )
                nc.tensor.matmul(
                    p0[:], wp[:, 1, 0:128], at[:, 1, :], start=False, stop=True
                )
                nc.tensor.matmul(
                    p1[:], wp[:, 0, 128:192], at[:, 0, :], start=True, stop=False
                )
                nc.tensor.matmul(
                    p1[:], wp[:, 1, 128:192], at[:, 1, :], start=False, stop=True
                )
                ot = outp.tile([128, 512], F32, tag="ot")
                ot2 = outp.tile([64, 512], F32, tag="ot2")
                nc.scalar.copy(ot[:], p0[:])
                nc.scalar.copy(ot2[:], p1[:])
                nc.sync.dma_start(outd[0:128, sl], ot[:])
                nc.sync.dma_start(outd[128:192, sl], ot2[:])


_NC_CACHE = {}


def _get_nc(dw_mode=None):
    key = dw_mode or DW_MODE
    if key not in _NC_CACHE:
        _NC_CACHE[key] = build(key)
    return _NC_CACHE[key]


def prep_inputs(x, w_qkv, w_dw, w_proj, temperature):
    x = np.asarray(x, np.float32)
    w_qkv = np.asarray(w_qkv, np.float32)
    w_dw = np.asarray(w_dw, np.float32).reshape(C3, 9)
    w_proj = np.asarray(w_proj, np.float32)
    temperature = np.asarray(temperature, np.float32).reshape(NH)

    wqT = np.ascontiguousarray(w_qkv.T)  # [192, 576]
    wq = np.zeros((128, 2, C3), np.float32)
    wq[:, 0, :] = wqT[0:128]
    wq[0:64, 1, :] = wqT[128:192]
    wq = wq.astype(ml_dtypes.bfloat16)

    wpT = np.ascontiguousarray(w_proj.T)  # [c, o]
    wp = np.zeros((96, 2, DIM), np.float32)
    wp[:, 0, :] = wpT[0:96]
    wp[:, 1, :] = wpT[96:192]
    wp = wp.astype(ml_dtypes.bfloat16)

    dww = np.zeros((128, NPB, 9), np.float32)
    for pb in range(NPB):
        sz = PB_SZ[pb]
        dww[:sz, pb, :] = w_dw[pb * 128 : pb * 128 + sz]

    idshift = np.zeros((128, 64), np.float32)
    idshift[64:128, :] = np.eye(64)
    idshift = idshift.astype(ml_dtypes.bfloat16)

    mask = np.full((96, 96), -1e30, np.float32)
    mask[0:48, 0:48] = 0.0
    mask[48:96, 48:96] = 0.0

    tmpv = np.ones((128, 3), np.float32)
    tmpv[:, 0] = temperature[np.arange(128) // CH]
    tmpv[0:64, 1] = temperature[(128 + np.arange(64)) // CH]

    maps = []
    for b in range(B):
        maps.append(
            {
                "x": np.ascontiguousarray(x[b].reshape(DIM, HW)).astype(ml_dtypes.bfloat16),
                "wq": wq,
                "wp": wp,
                "dww": dww,
                "tmpv": tmpv,
                "mask": mask,
                "idshift": idshift,
            }
        )
    return maps


def kernel(x, w_qkv, w_dw, w_proj, temperature, trace=False, tmpdir=None,
           dw_mode=None):
    nc = _get_nc(dw_mode)
    maps = prep_inputs(x, w_qkv, w_dw, w_proj, temperature)
    res = run_bass_kernel_spmd(
        nc, maps, core_ids=list(range(B)), trace=trace, tmpdir=tmpdir
    )
    out = np.stack(
        [np.asarray(r["out"]).reshape(DIM, H, W) for r in res.results]
    ).astype(np.float32)
    kernel.last_exec_time_ns = res.exec_time_ns
    return out


if __name__ == "__main__":
    nc = build()
    print("build ok")
